# revision 2
# baseline (speedup 1.0000x reference)
"""Trainium2 Bass kernel v2 for nn_CrossAttention (B=8, C=192, H=W=128, NH=4).

Data-parallel: 1 batch/core, no collectives. Differences vs v1 baseline:
  - Combined norm scale: qs = q * 1/sqrt(ssq_q*ssq_k) applied to q only; k is
    written unscaled (identical math: attn = (q/|q|)@(k/|k|)^T).
  - Single fused tensor_tensor writes the padded qkn tile [q*rs | k*1] using a
    broadcast rsx vector whose k-groups are preset to 1.0.
  - Persistent manual rings instead of per-iter memsets (ones rows for bias
    folding, pad zeros, rsx 1.0 groups are each written once).
  - Stage B runs out = (Wp@attn) @ v directly: M^T = (attn^T @ Wp^T) is built
    on-chip with 8 tiny matmuls after softmax, eliminating the attn@v
    intermediate (PSUM->SBUF copies and 64 matmuls).
  - Output stored as bf16 (halves write traffic); host upcasts to fp32.
  - Elementwise work spread across DVE/Pool/Act engines.
"""
import numpy as np
import ml_dtypes

_bf = ml_dtypes.bfloat16

B, C, H, W = 8, 192, 128, 128
NH = 4
HD = C // NH          # 48 head dim
P0 = 96               # channel half-group (2 k-tiles of 96 over C=192)
NG = 2                # head-pair groups
SP_FULL = H * W       # 16384


def _build_program(nc, s4, SP, CH=512):
    import concourse.tile as tile
    from concourse import mybir

    f32 = mybir.dt.float32
    bft = mybir.dt.bfloat16
    DB = 128
    NBLK = CH // DB          # 4 blocks per chunk
    NCH = SP // CH           # 32 chunks
    X = mybir.AxisListType.X
    ADD = mybir.AluOpType.add
    MAX = mybir.AluOpType.max
    MULT = mybir.AluOpType.mult
    EXP = mybir.ActivationFunctionType.Exp
    IDENT = mybir.ActivationFunctionType.Identity

    xq = nc.dram_tensor("xq", [C, SP], bft, kind="ExternalInput")
    xk = nc.dram_tensor("xk", [C, SP], bft, kind="ExternalInput")
    wqk = nc.dram_tensor("wqk", [97, 2, 2 * C], bft, kind="ExternalInput")
    wv = nc.dram_tensor("wv", [97, 2, 256], bft, kind="ExternalInput")
    wp = nc.dram_tensor("wp", [128, 2, C], bft, kind="ExternalInput")
    bp2 = nc.dram_tensor("bp2", [P0, 2], f32, kind="ExternalInput")
    out = nc.dram_tensor("out", [C, SP], bft, kind="ExternalOutput")

    xq_r = xq.ap().rearrange("(t p) d -> p t d", p=P0)
    xk_r = xk.ap().rearrange("(t p) d -> p t d", p=P0)
    out_r = out.ap().rearrange("(t p) d -> p t d", p=P0)

    NXB = 3    # x input ring depth
    NQN = 3    # qkn group-tile ring depth
    NSQ = 2
    NOB = 3

    with tile.TileContext(nc) as tc:
        with tc.tile_pool(name="const", bufs=1) as cpool:
            wqk_sb = cpool.tile([97, 2, 2 * C], bft)
            nc.sync.dma_start(wqk_sb[:], wqk.ap())
            wv_sb = cpool.tile([97, 2, 256], bft)
            nc.sync.dma_start(wv_sb[:], wv.ap())
            wp_sb = cpool.tile([128, 2, C], bft)
            nc.sync.dma_start(wp_sb[:], wp.ap())
            bp_sb = cpool.tile([P0, 2], f32)
            nc.sync.dma_start(bp_sb[:], bp2.ap())

            v_sb = cpool.tile([128, NG, SP], bft)
            mT_sb = cpool.tile([128, NG, C], bft)
            nc.gpsimd.memset(mT_sb[:], 0.0)

            # persistent rings; constant regions written once
            xq_bufs, xk_bufs = [], []
            for r in range(NXB):
                tq = cpool.tile([97, 2, CH], bft, name=f"xq_b{r}")
                nc.gpsimd.memset(tq[P0:97], 1.0)
                xq_bufs.append(tq)
                tk = cpool.tile([97, 2, CH], bft, name=f"xk_b{r}")
                nc.gpsimd.memset(tk[P0:97], 1.0)
                xk_bufs.append(tk)
            # qkn group tile: NBLK blocks x padded [q h0..h3 | k h0..h3] zones
            qkn_bufs = []
            for r in range(NQN):
                t = cpool.tile([128, NBLK, 512], bft, name=f"qkn_b{r}")
                nc.gpsimd.memset(
                    t.rearrange("p i (g c) -> p i g c", c=64)[:, :, :, HD:64], 0.0
                )
                qkn_bufs.append(t)
            sq_bufs = [cpool.tile([128, NBLK, 8, HD], bft, name=f"sq_b{r}")
                       for r in range(NSQ)]
            ss_bufs = [cpool.tile([128, NBLK, 8], f32, name=f"ss_b{r}")
                       for r in range(NSQ)]
            ssc_bufs = [cpool.tile([128, NBLK, 4], f32, name=f"ssc_b{r}")
                        for r in range(NSQ)]
            rsx_bufs = [cpool.tile([128, NBLK, 4], f32, name=f"rsx_b{r}")
                        for r in range(NSQ)]
            out_bufs = [cpool.tile([P0, NG, CH], bft, name=f"out_b{r}")
                        for r in range(NOB)]

            # ---------------- Stage A: proj, norm, attn accumulation
            with tc.tile_pool(name="npool", bufs=6) as npool, \
                 tc.tile_pool(name="qk_ps", bufs=3, space="PSUM") as qk_ps, \
                 tc.tile_pool(name="v_ps", bufs=2, space="PSUM") as v_ps, \
                 tc.tile_pool(name="at_ps", bufs=1, space="PSUM") as at_ps:

                attn_ps = []
                for g in range(NG):
                    attn_g = at_ps.tile([128, 128], f32, name=f"attn_g{g}",
                                        tag=f"attn{g}")
                    attn_ps.append(attn_g)

                for j in range(NCH):
                    sl = slice(j * CH, (j + 1) * CH)
                    xq_t = xq_bufs[j % NXB]
                    nc.sync.dma_start(xq_t[0:P0], xq_r[:, :, sl])
                    xk_t = xk_bufs[j % NXB]
                    nc.sync.dma_start(xk_t[0:P0], xk_r[:, :, sl])

                    # v projection into padded 128-row layout
                    for g in range(NG):
                        v_psum = v_ps.tile([128, CH], f32, tag="v", name="v_psum")
                        for kt in range(2):
                            nc.tensor.matmul(
                                v_psum[:],
                                lhsT=wv_sb[:, kt, g * 128:(g + 1) * 128],
                                rhs=xk_t[:, kt, :],
                                start=(kt == 0), stop=(kt == 1),
                            )
                        if g == 0:
                            nc.scalar.copy(v_sb[:, g, sl], v_psum[:])
                        else:
                            nc.vector.tensor_copy(v_sb[:, g, sl], v_psum[:])

                    qkn = qkn_bufs[j % NQN]
                    qkn_z = qkn.rearrange("p i (g c) -> p i g c", c=64)
                    for i in range(NBLK):
                        bsl = slice(i * DB, (i + 1) * DB)
                        qk_psum = qk_ps.tile([DB, 2 * C], f32, tag="qk",
                                             name="qk_psum")
                        for kt in range(2):
                            nc.tensor.matmul(
                                qk_psum[:, 0:C],
                                lhsT=xq_t[:, kt, bsl],
                                rhs=wqk_sb[:, kt, 0:C],
                                start=(kt == 0), stop=(kt == 1),
                            )
                        for kt in range(2):
                            nc.tensor.matmul(
                                qk_psum[:, C:2 * C],
                                lhsT=xk_t[:, kt, bsl],
                                rhs=wqk_sb[:, kt, C:2 * C],
                                start=(kt == 0), stop=(kt == 1),
                            )
                        # unscaled copy into padded zones (releases psum);
                        # alternate engines to balance load
                        dst = qkn_z[:, i, :, 0:HD]
                        src = qk_psum.rearrange("p (g c) -> p g c", c=HD)
                        if i % 4 == 3:
                            nc.vector.tensor_copy(dst, src)
                        else:
                            nc.scalar.copy(dst, src)
                    # batched norm math over all NBLK blocks (bf16 squares)
                    sq = sq_bufs[j % NSQ]
                    nc.vector.tensor_tensor(
                        out=sq[:], in0=qkn_z[:, :, :, 0:HD],
                        in1=qkn_z[:, :, :, 0:HD], op=MULT,
                    )
                    ss = ss_bufs[j % NSQ]
                    nc.vector.tensor_reduce(out=ss[:], in_=sq[:], axis=X, op=ADD)
                    ssc = ssc_bufs[j % NSQ]
                    nc.vector.tensor_tensor(
                        out=ssc[:], in0=ss[:, :, 0:4], in1=ss[:, :, 4:8], op=MULT,
                    )
                    rq = npool.tile([128, NBLK, 4], f32, tag="rq", name="rq")
                    nc.vector.reciprocal(rq[:], ssc[:])
                    rsx = rsx_bufs[j % NSQ]
                    nc.scalar.sqrt(rsx[:], rq[:])
                    # in-place scale of q zones only
                    nc.gpsimd.tensor_tensor(
                        out=qkn_z[:, :, 0:4, 0:HD],
                        in0=qkn_z[:, :, 0:4, 0:HD],
                        in1=rsx[:, :, :, None].to_broadcast((128, NBLK, 4, HD)),
                        op=MULT,
                    )
                    for i in range(NBLK):
                        first = (j == 0 and i == 0)
                        last = (j == NCH - 1 and i == NBLK - 1)
                        for g in range(NG):
                            nc.tensor.matmul(
                                attn_ps[g][:],
                                lhsT=qkn[:, i, g * 128:(g + 1) * 128],
                                rhs=qkn[:, i, 256 + g * 128: 256 + (g + 1) * 128],
                                start=first, stop=last,
                            )

                # ---------------- softmax + M^T = attn^T @ Wp^T (tiny)
                for g in range(NG):
                    sm_sb = npool.tile([128, 128], bft, tag="sm", name="sm_sb")
                    mx = npool.tile([128, 1], f32, tag="mx", name="mx")
                    nb = npool.tile([128, 1], f32, tag="nb", name="nb")
                    ex = npool.tile([128, 128], f32, tag="ex", name="ex")
                    sme = npool.tile([128, 1], f32, tag="sme", name="sme")
                    rcp = npool.tile([128, 1], f32, tag="rcp", name="rcp")
                    for hs in range(2):
                        rsl = slice(64 * hs, 64 * hs + HD)
                        ablk = attn_ps[g][rsl, rsl]
                        s_h = float(s4[2 * g + hs])
                        nc.vector.tensor_reduce(out=mx[rsl], in_=ablk, axis=X,
                                                op=MAX)
                        nc.vector.tensor_scalar_mul(nb[rsl], mx[rsl], -s_h)
                        nc.scalar.activation(
                            out=ex[rsl, rsl], in_=ablk, func=EXP,
                            scale=s_h, bias=nb[rsl], accum_out=sme[rsl],
                        )
                        nc.vector.reciprocal(rcp[rsl], sme[rsl])
                        nc.vector.tensor_scalar_mul(sm_sb[rsl, rsl],
                                                    ex[rsl, rsl], rcp[rsl])
                    m_ps = at_ps.tile([128, C], f32, tag="mps", name="m_ps")
                    for hs in range(2):
                        rsl = slice(64 * hs, 64 * hs + HD)
                        nc.tensor.matmul(
                            m_ps[rsl, :],
                            lhsT=sm_sb[rsl, rsl],
                            rhs=wp_sb[rsl, g, :],
                            start=True, stop=True,
                        )
                        nc.vector.tensor_copy(mT_sb[rsl, g, :], m_ps[rsl, :])

            # ---------------- Stage B: out = M @ v + bp
            with tc.tile_pool(name="pp_ps", bufs=3, space="PSUM") as pp_ps:
                for j in range(NCH):
                    sl = slice(j * CH, (j + 1) * CH)
                    out_t = out_bufs[j % NOB]
                    for m in range(NG):
                        pp = pp_ps.tile([P0, CH], f32, tag="pp", name="pp")
                        for g in range(NG):
                            nc.tensor.matmul(
                                pp[:],
                                lhsT=mT_sb[:, g, m * P0:(m + 1) * P0],
                                rhs=v_sb[:, g, sl],
                                start=(g == 0), stop=(g == 1),
                            )
                        nc.scalar.activation(
                            out_t[:, m, :], pp[:],
                            func=IDENT, bias=bp_sb[:, m:m + 1], scale=1.0,
                        )
                    nc.sync.dma_start(out_r[:, :, sl], out_t[:])


def _host_weights(Wq, bq, Wkv, bkv, Wp, bp):
    wqk = np.zeros((97, 2, 2 * C), np.float32)
    wv = np.zeros((97, 2, 256), np.float32)
    wp_a = np.zeros((128, 2, C), np.float32)
    for kt in range(2):
        rows = slice(kt * P0, (kt + 1) * P0)
        wqk[0:P0, kt, 0:C] = Wq[:, rows].T
        wqk[0:P0, kt, C:2 * C] = Wkv[0:C, rows].T
        for g in range(NG):
            # v output channels in padded layout: head A at +0, head B at +64
            chA = slice(C + 96 * g, C + 96 * g + HD)
            chB = slice(C + 96 * g + HD, C + 96 * g + 2 * HD)
            wv[0:P0, kt, g * 128 + 0: g * 128 + HD] = Wkv[chA, rows].T
            wv[0:P0, kt, g * 128 + 64: g * 128 + 64 + HD] = Wkv[chB, rows].T
        # Wp contraction rows in padded layout
        wp_a[0:HD, kt, :] = Wp[:, kt * P0: kt * P0 + HD].T
        wp_a[64:64 + HD, kt, :] = Wp[:, kt * P0 + HD: kt * P0 + 2 * HD].T
    wqk[P0, 0, 0:C] = bq
    wqk[P0, 0, C:2 * C] = bkv[0:C]
    for g in range(NG):
        wv[P0, 0, g * 128 + 0: g * 128 + HD] = bkv[C + 96 * g: C + 96 * g + HD]
        wv[P0, 0, g * 128 + 64: g * 128 + 64 + HD] = bkv[C + 96 * g + HD: C + 96 * g + 2 * HD]
    bp2 = bp.reshape(2, P0).T.copy()   # bp2[p, m] = bp[m*96+p]
    return wqk.astype(_bf), wv.astype(_bf), wp_a.astype(_bf), bp2


_PROG_CACHE = {}


def _get_prog(s4, SP):
    key = (tuple(np.asarray(s4, np.float64).tolist()), SP)
    if key not in _PROG_CACHE:
        import concourse.bacc as bacc
        nc = bacc.Bacc("TRN2", target_bir_lowering=False, debug=False, num_devices=B)
        _build_program(nc, s4, SP)
        nc.compile()
        _PROG_CACHE[key] = nc
    return _PROG_CACHE[key]


def make_in_maps(inputs, SP=SP_FULL):
    x_q = np.asarray(inputs["x_q"], np.float32)
    x_k = np.asarray(inputs["x_k"], np.float32)
    temp = np.asarray(inputs["temperature"], np.float32).reshape(-1)
    s4 = (0.1 / (1.0 + np.exp(-temp))).astype(np.float64)
    wqk, wv, wp_a, bp2 = _host_weights(
        np.asarray(inputs["Wq"], np.float32), np.asarray(inputs["bq"], np.float32),
        np.asarray(inputs["Wkv"], np.float32), np.asarray(inputs["bkv"], np.float32),
        np.asarray(inputs["Wp"], np.float32), np.asarray(inputs["bp"], np.float32),
    )
    nb = x_q.shape[0]
    in_maps = []
    for b in range(nb):
        in_maps.append({
            "xq": np.ascontiguousarray(x_q[b].reshape(C, SP).astype(_bf)),
            "xk": np.ascontiguousarray(x_k[b].reshape(C, SP).astype(_bf)),
            "wqk": wqk, "wv": wv, "wp": wp_a, "bp2": bp2,
        })
    return in_maps, s4


def _make_runner(nc, n_cores):
    """Reusable sharded PJRT callable for nc (mirrors bass2jax.run_bass_via_pjrt)."""
    import jax
    from jax.sharding import Mesh, PartitionSpec
    from jax.experimental.shard_map import shard_map
    from concourse import mybir
    from concourse.bass2jax import (
        _bass_exec_p, install_neuronx_cc_hook, partition_id_tensor,
    )

    install_neuronx_cc_hook()
    partition_name = nc.partition_id_tensor.name if nc.partition_id_tensor else None
    in_names, out_names, out_avals, zero_outs = [], [], [], []
    for alloc in nc.m.functions[0].allocations:
        if not isinstance(alloc, mybir.MemoryLocationSet):
            continue
        name = alloc.memorylocations[0].name
        if alloc.kind == "ExternalInput":
            if name != partition_name:
                in_names.append(name)
        elif alloc.kind == "ExternalOutput":
            out_names.append(name)
            shape = tuple(alloc.tensor_shape)
            dtype = mybir.dt.np(alloc.dtype)
            out_avals.append(jax.core.ShapedArray(shape, dtype))
            zero_outs.append(np.zeros(shape, dtype))
    n_params = len(in_names)
    all_in = in_names + out_names
    if partition_name is not None:
        all_in = all_in + [partition_name]
    all_in = tuple(all_in)

    def _body(*args):
        operands = list(args)
        if partition_name is not None:
            operands.append(partition_id_tensor())
        outs = _bass_exec_p.bind(
            *operands, out_avals=tuple(out_avals), in_names=all_in,
            out_names=tuple(out_names), lowering_input_output_aliases=(),
            sim_require_finite=True, sim_require_nnan=True, nc=nc,
        )
        return tuple(outs)

    devices = jax.devices()[:n_cores]
    mesh = Mesh(np.asarray(devices), ("core",))
    in_specs = (PartitionSpec("core"),) * (n_params + len(out_names))
    out_specs = (PartitionSpec("core"),) * len(out_names)
    fn = jax.jit(
        shard_map(_body, mesh=mesh, in_specs=in_specs, out_specs=out_specs,
                  check_rep=False),
        keep_unused=True,
    )
    return fn, in_names, out_names, zero_outs, mesh


_RUNNER_CACHE = {}


def _get_runner(s4, SP=SP_FULL):
    key = (tuple(np.asarray(s4, np.float64).tolist()), SP)
    if key not in _RUNNER_CACHE:
        nc = _get_prog(s4, SP)
        _RUNNER_CACHE[key] = _make_runner(nc, B)
    return _RUNNER_CACHE[key]


def _concat_args(in_maps, in_names, zero_outs):
    args = [np.concatenate([np.asarray(m[n]) for m in in_maps], axis=0)
            for n in in_names]
    for z in zero_outs:
        args.append(np.zeros((len(in_maps) * z.shape[0], *z.shape[1:]), z.dtype))
    return args


def kernel(**inputs):
    in_maps, s4 = make_in_maps(inputs)
    fn, in_names, out_names, zero_outs, mesh = _get_runner(s4)
    args = _concat_args(in_maps, in_names, zero_outs)
    out = fn(*args)
    o = np.asarray(out[out_names.index("out")])
    return o.reshape(B, C, H, W).astype(np.float32)


def bench(inputs, iters=30):
    """Return (min_per_iter_ns, mean_ns) for the 8-core dispatch with
    device-resident inputs (amortizes host->device transfer)."""
    import jax
    import time as _time
    from jax.sharding import NamedSharding, PartitionSpec
    in_maps, s4 = make_in_maps(inputs)
    fn, in_names, out_names, zero_outs, mesh = _get_runner(s4)
    sh = NamedSharding(mesh, PartitionSpec("core"))
    args = [jax.device_put(a, sh) for a in _concat_args(in_maps, in_names, zero_outs)]
    out = fn(*args)
    jax.block_until_ready(out)
    times = []
    for _ in range(iters):
        t0 = _time.perf_counter()
        out = fn(*args)
        jax.block_until_ready(out)
        times.append(_time.perf_counter() - t0)
    return min(times) * 1e9, (sum(times) / len(times)) * 1e9
